# revision 6
# baseline (speedup 1.0000x reference)
"""Trainium2 Bass kernel for the one-hot Conv2DProduct.

Math: the reference is a VALID conv, stride (2,2), kernel 2x2, with a one-hot
HWIO weight where output channel o selects input channel (o // 32**k) % 32 at
kernel cell k (row-major cells).  With C_OUT = 512 < 32**2, cells 2 and 3
always select channel 0, so

  out[b, i, j, o] = x[b, 2i, 2j,   o % 32]      (cell 0)
                  + x[b, 2i, 2j+1, o // 32]     (cell 1; o//32 < 16)
                  + x[b, 2i+1, 2j,   0]         (cell 2)
                  + x[b, 2i+1, 2j+1, 0]         (cell 3)

Per output pixel this is v @ M for a 50-vector v = [32 A-channels, 16
B-channels, pl0, pl1] and a fixed matrix M[50, 512] (one-hot rows + two
all-ones rows).  The kernel runs it on TensorE: per (group, j) tile the
stationary operand is the host-packed v-vectors of 128 output rows [50, 128],
the moving operand is M (resident in SBUF), accumulating f32 into one PSUM
bank.  Consecutive j alternate PE row strips (SBUF partitions 0-49 vs
64-113) so each LDWEIGHTS targets a row group disjoint from the in-flight
matmul -- the PE pulls it ahead and runs neighbouring matmuls concurrently
on disjoint sub-arrays instead of serializing LDW -> MM -> LDW.

ACT and DVE drain each 4-bank PSUM quad into bf16 SBUF tiles, split 1216:832
to equalize their per-element rates (ACT ~0.80 ns/elem vs DVE ~1.07, and DVE
carries more semaphore traffic).  All input loads are prefetched up front on
the SP HWDGE ring in half-group chunks (first matmul starts ~3 us in); the
output stores are issued on the same ring behind them, with a width ramp
(4,4,8,16,... j-columns) so the store stream starts early and ends promptly.

Everything is bf16 end to end (the harness gate is rel_err < 2e-2; measured
bf16 error is ~5e-3): the f32 baseline at 221 us was pinned to the ~358 GB/s
per-NeuronCore HBM limit by its 67 MB f32 store stream, so halving the bytes
halves the roofline.  Data-parallel over batch across the 8 cores; the host
re-layout (pure gather/cast) keeps per-core input DMA at 4.2 MB.
"""

import sys

import numpy as np

_REPO = "/opt/trn_rl_repo"
if _REPO not in sys.path:
    sys.path.insert(0, _REPO)

import ml_dtypes

import concourse.bacc as bacc
import concourse.mybir as mybir
from concourse import tile
from concourse.bass_utils import run_bass_kernel_spmd

B, H, W, C = 64, 128, 128, 32
OH, OW, CO = 64, 64, 512
N_CORES = 8
B_LOC = B // N_CORES  # batches per core
F32 = mybir.dt.float32
BF16 = mybir.dt.bfloat16
KF = 50  # features per output pixel: 32 A + 16 B + 2 odd-row values
G, P = 4, 128  # B_LOC*OH = 512 output rows as 4 groups of 128 partitions
STRIP = 64  # partition base of the second PE row strip
ACT_SPLIT = 1216  # of each 2048-elem PSUM quad, ACT drains [0:1216], DVE the rest

# store widths (j-columns) per group: ramp up early, ramp down at the end
_W16 = [16, 16, 16, 16]
WIDTHS = {0: [4, 4, 8, 16, 16, 16], G - 1: [16, 16, 16, 8, 4, 4]}


def _mat():
    o = np.arange(CO)
    m = np.zeros((KF, CO), dtype=np.float32)
    m[o % C, o] = 1.0
    m[C + o // C, o] = 1.0
    m[C + 16, :] = 1.0
    m[C + 17, :] = 1.0
    return m


def make_mat2():
    """Moving operand, duplicated at both PE row-strip bases: [128, CO]."""
    m2 = np.zeros((2 * STRIP, CO), dtype=np.float32)
    m = _mat()
    m2[0:KF] = m
    m2[STRIP:STRIP + KF] = m
    return m2.astype(ml_dtypes.bfloat16)


def pack_inputs(x_local):
    """[b, H, W, C] f32 -> {"xt": [128, G, OW//2, P] bf16, "mat": [128, CO]}.

    xt[k, g, jp, p] holds feature k of output pixel (row g*128+p, column
    2*jp) in partitions 0..49 and of column 2*jp+1 in partitions 64..113
    (rows ordered batch-major, then i) -- stationary operands for the two
    alternating PE row strips.
    """
    feats = np.empty((x_local.shape[0], OH, OW, KF), dtype=np.float32)
    feats[..., 0:C] = x_local[:, 0::2, 0::2, :]
    feats[..., C:C + 16] = x_local[:, 0::2, 1::2, :16]
    feats[..., C + 16] = x_local[:, 1::2, 0::2, 0]
    feats[..., C + 17] = x_local[:, 1::2, 1::2, 0]
    # [G, P, OW//2, 2, KF] -> strip-major [2, KF, G, OW//2, P]
    ft = feats.reshape(G, P, OW // 2, 2, KF).transpose(3, 4, 0, 2, 1)
    xt = np.zeros((2 * STRIP, G, OW // 2, P), dtype=np.float32)
    xt[0:KF] = ft[0]
    xt[STRIP:STRIP + KF] = ft[1]
    return {
        "xt": np.ascontiguousarray(xt.astype(ml_dtypes.bfloat16)),
        "mat": make_mat2(),
    }


def build_bass():
    nc = bacc.Bacc("TRN2", target_bir_lowering=False, debug=False)
    xt_d = nc.dram_tensor("xt", [2 * STRIP, G, OW // 2, P], BF16, kind="ExternalInput")
    mat_d = nc.dram_tensor("mat", [2 * STRIP, CO], BF16, kind="ExternalInput")
    out = nc.dram_tensor("out", [B_LOC, OH, OW, CO], BF16, kind="ExternalOutput")
    HG = (OW // 2) // 2  # jp columns per half-group load chunk

    with tile.TileContext(nc) as tc:
        with (
            tc.tile_pool(name="const", bufs=1) as cpool,
            tc.tile_pool(name="inp", bufs=1) as in_pool,
            tc.tile_pool(name="ps", bufs=2, space="PSUM") as ps_pool,
            tc.tile_pool(name="outp", bufs=3) as out_pool,
        ):
            out_d = out[:].rearrange("b i j o -> (b i) (j o)")
            mat_s = cpool.tile([2 * STRIP, CO], BF16, name="mat")
            nc.scalar.dma_start(mat_s[:], mat_d[:])

            # Prefetch every input chunk up front (SP ring is otherwise idle;
            # the store stream queues behind these on the same FIFO ring).
            xts = {}
            for g in range(G):
                for h in range(2):
                    t = in_pool.tile([2 * STRIP, HG * P], BF16, name=f"xt{g}_{h}")
                    nc.scalar.dma_start(
                        t[:],
                        xt_d[:, g, h * HG:(h + 1) * HG].rearrange("k j p -> k (j p)"),
                    )
                    xts[g, h] = t.rearrange("k (j p) -> k j p", p=P)

            for g in range(G):
                psl = slice(g * P, (g + 1) * P)
                j0 = 0
                for w in WIDTHS.get(g, _W16):
                    ot = out_pool.tile([P, 16 * CO], BF16, name=f"ot{g}_{j0}", tag="ot")
                    for q in range(w // 4):
                        pt = ps_pool.tile([P, 4 * CO], F32, name=f"pt{g}_{j0}_{q}", tag="pt")
                        for jj in range(4):
                            j = j0 + q * 4 + jj
                            jp = j // 2
                            sb = (j % 2) * STRIP  # alternate PE row strips
                            xr = xts[g, jp // HG]
                            nc.tensor.matmul(
                                pt[:, jj * CO:(jj + 1) * CO],
                                xr[sb:sb + KF, jp % HG, :],
                                mat_s[sb:sb + KF, :],
                                start=True,
                                stop=True,
                            )
                        # Drain the quad: ACT takes [0:ACT_SPLIT], DVE the rest.
                        base = (q * 4) * CO
                        nc.scalar.copy(ot[:, base:base + ACT_SPLIT], pt[:, 0:ACT_SPLIT])
                        nc.vector.tensor_copy(
                            ot[:, base + ACT_SPLIT:base + 4 * CO], pt[:, ACT_SPLIT:4 * CO]
                        )
                    nc.sync.dma_start(
                        out_d[psl, j0 * CO:(j0 + w) * CO], ot[:, 0:w * CO]
                    )
                    j0 += w
    return nc


_NC = None


def _get_nc():
    global _NC
    if _NC is None:
        _NC = build_bass()
        _NC.compile()  # bacc register allocation + lowering
    return _NC


def kernel(**inputs):
    x = np.ascontiguousarray(np.asarray(inputs["x"], dtype=np.float32))
    assert x.shape == (B, H, W, C), x.shape
    nc = _get_nc()
    in_maps = [pack_inputs(x[c * B_LOC:(c + 1) * B_LOC]) for c in range(N_CORES)]
    res = run_bass_kernel_spmd(nc, in_maps, list(range(N_CORES))).results
    return np.concatenate(
        [np.asarray(r["out"]).astype(np.float32) for r in res], axis=0
    )


# revision 7
# speedup vs baseline: 1.0107x; 1.0107x over previous
"""Trainium2 Bass kernel for the one-hot Conv2DProduct.

Math: the reference is a VALID conv, stride (2,2), kernel 2x2, with a one-hot
HWIO weight where output channel o selects input channel (o // 32**k) % 32 at
kernel cell k (row-major cells).  With C_OUT = 512 < 32**2, cells 2 and 3
always select channel 0, so

  out[b, i, j, o] = x[b, 2i, 2j,   o % 32]      (cell 0)
                  + x[b, 2i, 2j+1, o // 32]     (cell 1; o//32 < 16)
                  + x[b, 2i+1, 2j,   0]         (cell 2)
                  + x[b, 2i+1, 2j+1, 0]         (cell 3)

Per output pixel this is v @ M for a 50-vector v = [32 A-channels, 16
B-channels, pl0, pl1] and a fixed matrix M[50, 512] (one-hot rows + two
all-ones rows).  The kernel runs it on TensorE: per (group, j) tile the
stationary operand is the host-packed v-vectors of 128 output rows [50, 128],
the moving operand is M (resident in SBUF), accumulating f32 into one PSUM
bank.  Consecutive j alternate PE row strips (SBUF partitions 0-49 vs
64-113) so each LDWEIGHTS targets a row group disjoint from the in-flight
matmul -- the PE pulls it ahead and runs neighbouring matmuls concurrently
on disjoint sub-arrays instead of serializing LDW -> MM -> LDW.

Each 4-bank PSUM quad (4 j-columns) is drained f32->bf16 by ACT (j0,j1) and
DVE (j2,j3) IN PARALLEL into separate SBUF tiles -- a shared tile makes the
Tile framework serialize the two writers, which costs ~30%.  The two tiles
stream out as interleaved stores (2 KiB HBM runs) on the otherwise-idle SP
HWDGE ring; input chunks prefetch on the ACT ring (smallest chunk first so
the first matmul starts early) and the tiny matrix on the SP ring.

Everything is bf16 end to end (the harness gate is rel_err < 2e-2; measured
bf16 error is ~5e-3): the f32 baseline at 221 us was pinned to the ~358 GB/s
per-NeuronCore HBM limit by its 67 MB f32 store stream, so halving the bytes
halves the roofline.  Data-parallel over batch across the 8 cores; the host
re-layout (pure gather/cast) keeps per-core input DMA at 4.2 MB.
"""

import sys

import numpy as np

_REPO = "/opt/trn_rl_repo"
if _REPO not in sys.path:
    sys.path.insert(0, _REPO)

import ml_dtypes

import concourse.bacc as bacc
import concourse.mybir as mybir
from concourse import tile
from concourse.bass_utils import run_bass_kernel_spmd

B, H, W, C = 64, 128, 128, 32
OH, OW, CO = 64, 64, 512
N_CORES = 8
B_LOC = B // N_CORES  # batches per core
F32 = mybir.dt.float32
BF16 = mybir.dt.bfloat16
KF = 50  # features per output pixel: 32 A + 16 B + 2 odd-row values
G, P = 4, 128  # B_LOC*OH = 512 output rows as 4 groups of 128 partitions
STRIP = 64  # partition base of the second PE row strip
HALF = 2 * CO  # per-quad drain share per engine (2 j-columns, j-aligned)

# store widths (j-columns) per group: ramp up early, ramp down at the end
_W16 = [16, 16, 16, 16]
WIDTHS = {0: [4, 4, 8, 16, 16, 16], G - 1: [16, 16, 16, 8, 4, 4]}
# input chunk sizes (jp-columns) per group: small first chunk starts MMs early
_C16 = [16, 16]
CHUNKS = {0: [4, 12, 16]}


def _mat():
    o = np.arange(CO)
    m = np.zeros((KF, CO), dtype=np.float32)
    m[o % C, o] = 1.0
    m[C + o // C, o] = 1.0
    m[C + 16, :] = 1.0
    m[C + 17, :] = 1.0
    return m


def make_mat2():
    """Moving operand, duplicated at both PE row-strip bases: [128, CO]."""
    m2 = np.zeros((2 * STRIP, CO), dtype=np.float32)
    m = _mat()
    m2[0:KF] = m
    m2[STRIP:STRIP + KF] = m
    return m2.astype(ml_dtypes.bfloat16)


def pack_inputs(x_local):
    """[b, H, W, C] f32 -> {"xt": [128, G, OW//2, P] bf16, "mat": [128, CO]}.

    xt[k, g, jp, p] holds feature k of output pixel (row g*128+p, column
    2*jp) in partitions 0..49 and of column 2*jp+1 in partitions 64..113
    (rows ordered batch-major, then i) -- stationary operands for the two
    alternating PE row strips.
    """
    feats = np.empty((x_local.shape[0], OH, OW, KF), dtype=np.float32)
    feats[..., 0:C] = x_local[:, 0::2, 0::2, :]
    feats[..., C:C + 16] = x_local[:, 0::2, 1::2, :16]
    feats[..., C + 16] = x_local[:, 1::2, 0::2, 0]
    feats[..., C + 17] = x_local[:, 1::2, 1::2, 0]
    # [G, P, OW//2, 2, KF] -> strip-major [2, KF, G, OW//2, P]
    ft = feats.reshape(G, P, OW // 2, 2, KF).transpose(3, 4, 0, 2, 1)
    xt = np.zeros((2 * STRIP, G, OW // 2, P), dtype=np.float32)
    xt[0:KF] = ft[0]
    xt[STRIP:STRIP + KF] = ft[1]
    return {
        "xt": np.ascontiguousarray(xt.astype(ml_dtypes.bfloat16)),
        "mat": make_mat2(),
    }


def build_bass():
    nc = bacc.Bacc("TRN2", target_bir_lowering=False, debug=False)
    xt_d = nc.dram_tensor("xt", [2 * STRIP, G, OW // 2, P], BF16, kind="ExternalInput")
    mat_d = nc.dram_tensor("mat", [2 * STRIP, CO], BF16, kind="ExternalInput")
    out = nc.dram_tensor("out", [B_LOC, OH, OW, CO], BF16, kind="ExternalOutput")

    with tile.TileContext(nc) as tc:
        with (
            tc.tile_pool(name="const", bufs=1) as cpool,
            tc.tile_pool(name="inp", bufs=1) as in_pool,
            tc.tile_pool(name="ps", bufs=2, space="PSUM") as ps_pool,
            tc.tile_pool(name="outa", bufs=3) as outa_pool,
            tc.tile_pool(name="outb", bufs=3) as outb_pool,
        ):
            out_d = out[:].rearrange("b i j o -> (b i) (j o)")
            mat_s = cpool.tile([2 * STRIP, CO], BF16, name="mat")
            nc.sync.dma_start(mat_s[:], mat_d[:])

            # Prefetch input chunks on the ACT ring, smallest first.
            xts = []  # per group: list of (jp_start, jp_view)
            for g in range(G):
                lst, jp0 = [], 0
                for ci, cw in enumerate(CHUNKS.get(g, _C16)):
                    t = in_pool.tile([2 * STRIP, cw * P], BF16, name=f"xt{g}_{ci}")
                    nc.scalar.dma_start(
                        t[:],
                        xt_d[:, g, jp0:jp0 + cw].rearrange("k j p -> k (j p)"),
                    )
                    lst.append((jp0, t.rearrange("k (j p) -> k j p", p=P)))
                    jp0 += cw
                xts.append(lst)

            def xt_view(g, jp):
                for jp0, v in reversed(xts[g]):
                    if jp >= jp0:
                        return v[:, jp - jp0, :]
                raise AssertionError

            for g in range(G):
                psl = slice(g * P, (g + 1) * P)
                j0 = 0
                for w in WIDTHS.get(g, _W16):
                    qn = w // 4
                    ota = outa_pool.tile([P, 4 * HALF], BF16, name=f"ota{g}_{j0}", tag="ota")
                    otb = outb_pool.tile([P, 4 * HALF], BF16, name=f"otb{g}_{j0}", tag="otb")
                    for q in range(qn):
                        pt = ps_pool.tile([P, 4 * CO], F32, name=f"pt{g}_{j0}_{q}", tag="pt")
                        for jj in range(4):
                            j = j0 + q * 4 + jj
                            jp = j // 2
                            sb = (j % 2) * STRIP  # alternate PE row strips
                            nc.tensor.matmul(
                                pt[:, jj * CO:(jj + 1) * CO],
                                xt_view(g, jp)[sb:sb + KF, :],
                                mat_s[sb:sb + KF, :],
                                start=True,
                                stop=True,
                            )
                        # Parallel drain into per-engine tiles (no shared-tile
                        # write serialization): ACT j0,j1 -- DVE j2,j3.
                        nc.scalar.copy(ota[:, q * HALF:(q + 1) * HALF], pt[:, 0:HALF])
                        nc.vector.tensor_copy(otb[:, q * HALF:(q + 1) * HALF], pt[:, HALF:2 * HALF])
                    # Interleaved stores: quad q covers j0+4q..j0+4q+3; ota has
                    # the first two j of each quad, otb the last two.
                    dw = out_d[psl, j0 * CO:(j0 + w) * CO].rearrange(
                        "p (q h o) -> p q h o", q=qn, h=2, o=HALF
                    )
                    sa = ota.rearrange("p (q o) -> p q o", o=HALF)
                    sb_ = otb.rearrange("p (q o) -> p q o", o=HALF)
                    nc.sync.dma_start(dw[:, :, 0, :], sa[:, 0:qn])
                    nc.sync.dma_start(dw[:, :, 1, :], sb_[:, 0:qn])
                    j0 += w
    return nc


_NC = None


def _get_nc():
    global _NC
    if _NC is None:
        _NC = build_bass()
        _NC.compile()  # bacc register allocation + lowering
    return _NC


def kernel(**inputs):
    x = np.ascontiguousarray(np.asarray(inputs["x"], dtype=np.float32))
    assert x.shape == (B, H, W, C), x.shape
    nc = _get_nc()
    in_maps = [pack_inputs(x[c * B_LOC:(c + 1) * B_LOC]) for c in range(N_CORES)]
    res = run_bass_kernel_spmd(nc, in_maps, list(range(N_CORES))).results
    return np.concatenate(
        [np.asarray(r["out"]).astype(np.float32) for r in res], axis=0
    )


# revision 9
# speedup vs baseline: 1.1169x; 1.1050x over previous
"""Trainium2 Bass kernel for the one-hot Conv2DProduct.

Math: the reference is a VALID conv, stride (2,2), kernel 2x2, with a one-hot
HWIO weight where output channel o selects input channel (o // 32**k) % 32 at
kernel cell k (row-major cells).  With C_OUT = 512 < 32**2, cells 2 and 3
always select channel 0, so

  out[b, i, j, o] = x[b, 2i, 2j,   o % 32]      (cell 0)
                  + x[b, 2i, 2j+1, o // 32]     (cell 1; o//32 < 16)
                  + x[b, 2i+1, 2j,   0]         (cell 2)
                  + x[b, 2i+1, 2j+1, 0]         (cell 3)

Per output pixel this is v @ M for a 50-vector v = [32 A-channels, 16
B-channels, pl0, pl1] and a fixed matrix M[50, 512] (one-hot rows + two
all-ones rows).  The kernel runs it on TensorE: per (group, j) tile the
stationary operand is the host-packed v-vectors of 128 output rows [50, 128],
the moving operand is M (resident in SBUF), accumulating f32 into one PSUM
bank.  Consecutive j alternate PE row strips (SBUF partitions 0-49 vs
64-113) so each LDWEIGHTS targets a row group disjoint from the in-flight
matmul -- the PE pulls it ahead and runs neighbouring matmuls concurrently
on disjoint sub-arrays instead of serializing LDW -> MM -> LDW.

Each 4-bank PSUM quad (4 j-columns) is drained f32->bf16 by ACT (j0,j1) and
DVE (j2,j3) IN PARALLEL into separate SBUF tiles -- a shared tile makes the
Tile framework serialize the two writers, which costs ~30%.  The two tiles
stream out as interleaved stores (2 KiB HBM runs) on the otherwise-idle SP
HWDGE ring; input chunks prefetch on the ACT ring (smallest chunk first so
the first matmul starts early) and the tiny matrix on the SP ring.

Everything is bf16 end to end (the harness gate is rel_err < 2e-2; measured
bf16 error is ~5e-3): the f32 baseline at 221 us was pinned to the ~358 GB/s
per-NeuronCore HBM limit by its 67 MB f32 store stream, so halving the bytes
halves the roofline.  Data-parallel over batch across the 8 cores; the host
re-layout (pure gather/cast) keeps per-core input DMA at 4.2 MB.
"""

import sys

import numpy as np

_REPO = "/opt/trn_rl_repo"
if _REPO not in sys.path:
    sys.path.insert(0, _REPO)

import ml_dtypes

import concourse.bacc as bacc
import concourse.mybir as mybir
from concourse import tile
from concourse.bass_utils import run_bass_kernel_spmd

B, H, W, C = 64, 128, 128, 32
OH, OW, CO = 64, 64, 512
N_CORES = 8
B_LOC = B // N_CORES  # batches per core
F32 = mybir.dt.float32
BF16 = mybir.dt.bfloat16
KF = 50  # features per output pixel: 32 A + 16 B + 2 odd-row values
G, P = 4, 128  # B_LOC*OH = 512 output rows as 4 groups of 128 partitions
STRIP = 64  # partition base of the second PE row strip
HALF = 2 * CO  # per-quad drain share per engine (2 j-columns, j-aligned)

# store widths (j-columns) per group: ramp up early, ramp down at the end
_W16 = [16, 16, 16, 16]
WIDTHS = {0: [4, 4, 8, 16, 16, 16], G - 1: [16, 16, 16, 8, 4, 4]}
# input chunk sizes (jp-columns) per group: small first chunk starts MMs early
_C16 = [16, 16]
CHUNKS = {0: [4, 12, 16]}


def _mat():
    o = np.arange(CO)
    m = np.zeros((KF, CO), dtype=np.float32)
    m[o % C, o] = 1.0
    m[C + o // C, o] = 1.0
    m[C + 16, :] = 1.0
    m[C + 17, :] = 1.0
    return m


def make_mat2():
    """Moving operand, duplicated at both PE row-strip bases: [128, CO]."""
    m2 = np.zeros((2 * STRIP, CO), dtype=np.float32)
    m = _mat()
    m2[0:KF] = m
    m2[STRIP:STRIP + KF] = m
    return m2.astype(ml_dtypes.bfloat16)


def pack_inputs(x_local):
    """[b, H, W, C] f32 -> {"xt": [128, G, OW//2, P] bf16, "mat": [128, CO]}.

    xt[k, g, jp, p] holds feature k of output pixel (row g*128+p, column
    2*jp) in partitions 0..49 and of column 2*jp+1 in partitions 64..113
    (rows ordered batch-major, then i) -- stationary operands for the two
    alternating PE row strips.
    """
    feats = np.empty((x_local.shape[0], OH, OW, KF), dtype=np.float32)
    feats[..., 0:C] = x_local[:, 0::2, 0::2, :]
    feats[..., C:C + 16] = x_local[:, 0::2, 1::2, :16]
    feats[..., C + 16] = x_local[:, 1::2, 0::2, 0]
    feats[..., C + 17] = x_local[:, 1::2, 1::2, 0]
    # [G, P, OW//2, 2, KF] -> strip-major [2, KF, G, OW//2, P]
    ft = feats.reshape(G, P, OW // 2, 2, KF).transpose(3, 4, 0, 2, 1)
    xt = np.zeros((2 * STRIP, G, OW // 2, P), dtype=np.float32)
    xt[0:KF] = ft[0]
    xt[STRIP:STRIP + KF] = ft[1]
    return {
        "xt": np.ascontiguousarray(xt.astype(ml_dtypes.bfloat16)),
        "mat": make_mat2(),
    }


def build_bass():
    nc = bacc.Bacc("TRN2", target_bir_lowering=False, debug=False)
    xt_d = nc.dram_tensor("xt", [2 * STRIP, G, OW // 2, P], BF16, kind="ExternalInput")
    mat_d = nc.dram_tensor("mat", [2 * STRIP, CO], BF16, kind="ExternalInput")
    out = nc.dram_tensor("out", [B_LOC, OH, OW, CO], BF16, kind="ExternalOutput")

    with tile.TileContext(nc) as tc:
        with (
            tc.tile_pool(name="const", bufs=1) as cpool,
            tc.tile_pool(name="inp", bufs=1) as in_pool,
            tc.tile_pool(name="psa", bufs=2, space="PSUM") as psa_pool,
            tc.tile_pool(name="psb", bufs=2, space="PSUM") as psb_pool,
            tc.tile_pool(name="outa", bufs=3) as outa_pool,
            tc.tile_pool(name="outb", bufs=3) as outb_pool,
        ):
            out_d = out[:].rearrange("b i j o -> (b i) (j o)")
            mat_s = cpool.tile([2 * STRIP, CO], BF16, name="mat")
            nc.sync.dma_start(mat_s[:], mat_d[:])

            # Prefetch input chunks on the ACT ring, smallest first.
            xts = []  # per group: list of (jp_start, jp_view)
            for g in range(G):
                lst, jp0 = [], 0
                for ci, cw in enumerate(CHUNKS.get(g, _C16)):
                    t = in_pool.tile([2 * STRIP, cw * P], BF16, name=f"xt{g}_{ci}")
                    nc.scalar.dma_start(
                        t[:],
                        xt_d[:, g, jp0:jp0 + cw].rearrange("k j p -> k (j p)"),
                    )
                    lst.append((jp0, t.rearrange("k (j p) -> k j p", p=P)))
                    jp0 += cw
                xts.append(lst)

            def xt_view(g, jp):
                for jp0, v in reversed(xts[g]):
                    if jp >= jp0:
                        return v[:, jp - jp0, :]
                raise AssertionError

            for g in range(G):
                psl = slice(g * P, (g + 1) * P)
                j0 = 0
                for w in WIDTHS.get(g, _W16):
                    qn = w // 4
                    ota = outa_pool.tile([P, 4 * HALF], BF16, name=f"ota{g}_{j0}", tag="ota")
                    otb = outb_pool.tile([P, 4 * HALF], BF16, name=f"otb{g}_{j0}", tag="otb")
                    for q in range(qn):
                        # Two independent 2-bank PSUM tiles per quad -- one per
                        # drain engine -- so ACT and DVE never share a consumer
                        # dependency (a shared tile serializes their drains).
                        pa = psa_pool.tile([P, HALF], F32, name=f"pa{g}_{j0}_{q}", tag="pa")
                        pb = psb_pool.tile([P, HALF], F32, name=f"pb{g}_{j0}_{q}", tag="pb")
                        for jj in range(4):
                            j = j0 + q * 4 + jj
                            jp = j // 2
                            sb = (j % 2) * STRIP  # alternate PE row strips
                            pt = pa if jj < 2 else pb
                            nc.tensor.matmul(
                                pt[:, (jj % 2) * CO:(jj % 2 + 1) * CO],
                                xt_view(g, jp)[sb:sb + KF, :],
                                mat_s[sb:sb + KF, :],
                                start=True,
                                stop=True,
                            )
                        nc.scalar.copy(ota[:, q * HALF:(q + 1) * HALF], pa[:])
                        nc.vector.tensor_copy(otb[:, q * HALF:(q + 1) * HALF], pb[:])
                    # Interleaved stores: quad q covers j0+4q..j0+4q+3; ota has
                    # the first two j of each quad, otb the last two.
                    dw = out_d[psl, j0 * CO:(j0 + w) * CO].rearrange(
                        "p (q h o) -> p q h o", q=qn, h=2, o=HALF
                    )
                    sa = ota.rearrange("p (q o) -> p q o", o=HALF)
                    sb_ = otb.rearrange("p (q o) -> p q o", o=HALF)
                    nc.sync.dma_start(dw[:, :, 0, :], sa[:, 0:qn])
                    nc.sync.dma_start(dw[:, :, 1, :], sb_[:, 0:qn])
                    j0 += w
    return nc


_NC = None


def _get_nc():
    global _NC
    if _NC is None:
        _NC = build_bass()
        _NC.compile()  # bacc register allocation + lowering
    return _NC


def kernel(**inputs):
    x = np.ascontiguousarray(np.asarray(inputs["x"], dtype=np.float32))
    assert x.shape == (B, H, W, C), x.shape
    nc = _get_nc()
    in_maps = [pack_inputs(x[c * B_LOC:(c + 1) * B_LOC]) for c in range(N_CORES)]
    res = run_bass_kernel_spmd(nc, in_maps, list(range(N_CORES))).results
    return np.concatenate(
        [np.asarray(r["out"]).astype(np.float32) for r in res], axis=0
    )
